# revision 14
# baseline (speedup 1.0000x reference)
"""Trainium2 Bass kernel for nn_CCL__69277822485245 (spectral conv via DCT/FFT).

Math: the reference's rFFT along W cancels into a circular 5-tap convolution,
and the DCT-II sandwich M @ diag(D[:,s]) @ D collapses into 5 dense 128x128
matrices G_s (precomputed on host). Per batch element:

    u_s[i, m, w] = sum_h G_s[m, h] x[i, h, w]                  (stage 1)
    out[o, m, n] = sum_{s,t,i} W[o,i,s,t] u_s[i, m, (n-t)%W] + bias[o]   (stage 2)

Sharding: data-parallel over batch B=8 across the 8 NeuronCores (1 each).

Layouts (per core):
  stage 1: per output column w, one matmul
      lhsT = xdup[h=128, di=128]    (x duplicated on the host so the output
                                     partition dim carries (d, i) pairs)
      rhs  = G^T[h=128, (s5, m64)]  (m in halves of 64 -> N=320; G s-order
                                     is [0,2,4,1,3] so each half's psum->u
                                     copy is a contiguous column slice)
      out  = psum[(d,i)=128, (sidx, m)]
      psum->SBUF casts split the halves: partitions 0-63 keep s={0,2,4}
      (slots 0..2), partitions 64-127 keep s={1,3} (slots 0..1), batched
      two w-columns per cast. SBUF u[(d,i), (slot, j, m)] -- j-major-of-m
      so stage-2 reads contiguous (j,m) runs.
  stage 2: for each t (same shift for both halves) and slot c:
      one K=128 matmul contracts (i, s=2c) on partitions 0-63 and
      (i, s=2c+1) on 64-127 simultaneously (c=2: K=64, s=4 only);
      15 sequential PSUM-accumulating passes, N = (j8, m64) = 512 contiguous.
      Bias added during the single per-block PSUM->SBUF evac (ScalarE).

DTYPE selects the matmul operand precision:
  "bf16": fastest (1 cyc/row + fast weight load), rel err ~ 3e-3
  "f32r": TF32-like (~2 cyc/row), rel err ~ 2e-4
  "f32" : exact fp32 (4 cyc/row), slowest
"""

import numpy as np

H = 128
W = 128
CI = 64
CO = 128
KH = 5
KW = 5
B = 8

MH = 64          # m-half processed per outer iteration
WB = 16          # w-block
HALO = 4         # extra back-columns for the t-shifts
WEXT = WB + HALO
NSLOT = 3        # s-slots per partition half (s = 2c + d)
JT = 8           # j-tile inside stage 2 (N = JT*MH = 512)

DTYPE = "bf16"

_PROG = None
_CONSTS = None
_RUN_OPTS = {}     # test harness may set e.g. {"trace": True, "trace_cores": [0]}
_LAST_RESULT = None


def _np_dt():
    if DTYPE == "bf16":
        import ml_dtypes
        return ml_dtypes.bfloat16
    return np.float32


def _build_consts():
    n = np.arange(H, dtype=np.float64)
    ang = np.pi * (2.0 * n[None, :] + 1.0) * n[:, None] / (2.0 * H)  # [k, h]
    D = 2.0 * np.cos(ang)
    wgt = np.where(n == 0, 0.5, 1.0)
    M = (np.cos(ang).T * wgt[None, :]) / (2.0 * H)                    # [m, k]
    G = np.stack([M @ (D[:, s:s + 1] * D) for s in range(KH)])        # [s, m, h]
    G = G[[0, 2, 4, 1, 3]]   # s-order so each half's psum->u copy is contiguous
    # rhs layout [h, (mh, sidx, ml)]: col = mh*320 + sidx*64 + ml
    GT = (G.transpose(2, 0, 1)                # [h, s, m]
            .reshape(H, KH, 2, MH)            # [h, s, mh, ml]
            .transpose(0, 2, 1, 3)            # [h, mh, s, ml]
            .reshape(H, KH * H))
    return np.ascontiguousarray(GT).astype(_np_dt())


def _build_program():
    import concourse.mybir as mybir
    import concourse.tile as tile
    from concourse import bacc

    f32 = mybir.dt.float32
    mmdt = {"bf16": mybir.dt.bfloat16,
            "f32r": mybir.dt.float32r,
            "f32": mybir.dt.float32}[DTYPE]

    nc = bacc.Bacc("TRN2", target_bir_lowering=False, debug=False,
                   enable_asserts=False, num_devices=B)
    NBLK = W // WB
    # x stored rolled by HALO along w (stored col s holds w = s - HALO mod W)
    # and chunked so stage 1 can start before the full x has landed.
    x_d = nc.dram_tensor("x", [H, W * 2 * CI], mmdt, kind="ExternalInput").ap()
    g_d = nc.dram_tensor("g", [H, KH * H], mmdt, kind="ExternalInput").ap()
    w_d = nc.dram_tensor("wt", [128, KW * NSLOT * CO], mmdt,
                         kind="ExternalInput").ap()
    b_d = nc.dram_tensor("bias", [CO, 1], f32, kind="ExternalInput").ap()
    # out stored (w, m)-major; host transposes back to (m, w)
    o_d = nc.dram_tensor("out", [CO, W * H], f32, kind="ExternalOutput").ap()
    o3 = o_d.rearrange("p (w m) -> p w m", m=H)

    with tile.TileContext(nc) as tc:
        with (
            tc.tile_pool(name="const", bufs=1) as cpool,
            tc.tile_pool(name="xch", bufs=1) as xpool,
            tc.tile_pool(name="u", bufs=2) as upool,
            tc.tile_pool(name="oacc", bufs=1) as opool,
            tc.tile_pool(name="ps1", bufs=2, space="PSUM") as ps1,
            tc.tile_pool(name="ps2", bufs=2, space="PSUM") as ps2,
        ):
            gt = cpool.tile([H, KH * H], mmdt)
            nc.sync.dma_start(gt[:], g_d)
            wt = cpool.tile([128, KW * NSLOT * CO], mmdt)
            nc.scalar.dma_start(wt[:], w_d)
            bt = cpool.tile([CO, 1], f32)
            nc.scalar.dma_start(bt[:], b_d)
            x_c = x_d.rearrange("p (b rest) -> p b rest", b=NBLK)
            xch = []
            for bk in range(NBLK):
                xc = xpool.tile([H, WB * 2 * CI], mmdt, tag=f"x{bk}")
                # alternate the two HWDGE queues so triggers + transfers overlap
                eng = nc.sync if bk % 2 == 0 else nc.scalar
                eng.dma_start(xc[:], x_c[:, bk, :])
                xch.append(xc[:].rearrange("p (w di) -> p w di", w=WB))

            import concourse.mybir as _mb

            def stage1_gen(mh, blk, out):
                """Yields after each j0-group (2 matmuls + 2 evac copies)."""
                u = upool.tile([128, NSLOT * WEXT * MH], mmdt)
                u4 = u[:].rearrange("p (c j m) -> p c j m", c=NSLOT, j=WEXT)
                out.append(u4)
                for j0 in range(0, WEXT, 2):
                    p1 = ps1.tile([128, 1024], f32)
                    for dj in range(2):
                        sc = (blk * WB + j0 + dj) % W   # stored col (pre-rolled)
                        nc.tensor.matmul(p1[:, dj * 512:dj * 512 + KH * MH],
                                         xch[sc // WB][:, sc % WB, :],
                                         gt[:, mh * KH * MH:(mh + 1) * KH * MH],
                                         start=True, stop=True)
                    pv = p1[:].rearrange("p (j s m) -> p j s m", j=2, s=8)
                    # psum s-order [0,2,4,1,3]: half0 cols 0:192, half1 192:320
                    # evac split across engines: DVE (half0) + Act (half1)
                    nc.vector.tensor_copy(
                        u4[0:64, :, j0:j0 + 2, :].transpose([0, 2, 1, 3]),
                        pv[0:64, :, 0:3, :])
                    nc.scalar.activation(
                        u4[64:128, 0:2, j0:j0 + 2, :].transpose([0, 2, 1, 3]),
                        pv[64:128, :, 3:5, :],
                        _mb.ActivationFunctionType.Identity)
                    # third copy: s=4 into the idle half-1 slot shifted one w
                    # back (u4[64:128, 2, j] := u_s4[i, j-1]), so each c=2
                    # matmul covers taps (t, t+1) at K=128 (t-pair packing).
                    # psum rows 64:128 hold the same s=4 columns; alternate
                    # engines per group to balance the extra ~300ns.
                    nj = min(2, (WEXT - 1) - j0)   # last group: only j0 fits
                    dst = u4[64:128, 2, j0 + 1:j0 + 1 + nj, :]
                    src = pv[64:128, 0:nj, 2:3, :].transpose([0, 2, 1, 3])
                    if (j0 // 2) % 2 == 0:
                        nc.vector.tensor_copy(dst, src)
                    else:
                        nc.scalar.activation(
                            dst, src, _mb.ActivationFunctionType.Identity)
                    yield

            # c=2 runs last so the s4-shift DMA has landed; taps paired
            # (t, t+1) for t in {0, 2}, t=4 alone at K=64.
            S2BLOCKS = ([(t, c, 128) for c in (0, 1) for t in range(KW)]
                        + [(0, 2, 128), (2, 2, 128), (4, 2, 64)])

            def stage2_gen(u4, oa3, mh, blk):
                """Yields after every 3 matmuls."""
                p2 = ps2.tile([128, 2 * 512], f32)
                nmm = 0
                for bi, (t, c, kk) in enumerate(S2BLOCKS):
                    lhsT2 = wt[0:kk, (t * NSLOT + c) * CO:
                               (t * NSLOT + c + 1) * CO]
                    start = bi == 0
                    stop = bi == len(S2BLOCKS) - 1
                    for jt in range(WB // JT):
                        # contiguous (j8, m64) = 512 elems
                        rhs2 = u4[0:kk, c,
                                  HALO - t + jt * JT:
                                  HALO - t + (jt + 1) * JT, :]
                        nc.tensor.matmul(
                            p2[:, jt * 512:(jt + 1) * 512], lhsT2, rhs2,
                            start=start, stop=stop)
                        nmm += 1
                        if nmm % 3 == 0:
                            yield
                # psum (jt, j, m) == (w, m) order matches oacc layout: no
                # transpose in the evac; alternate engines by tile parity so
                # the 1.1us evac doesn't bunch either queue at tile boundaries
                if (mh * NBLK + blk) % 2 == 0:
                    nc.vector.tensor_scalar_add(
                        oa3[:, blk * WB:(blk + 1) * WB, :],
                        p2[:].rearrange("p (w m) -> p w m", m=MH),
                        bt[:])
                else:
                    nc.scalar.activation(
                        oa3[:, blk * WB:(blk + 1) * WB, :],
                        p2[:].rearrange("p (w m) -> p w m", m=MH),
                        _mb.ActivationFunctionType.Identity, bias=bt[:])
                # stream this tile's finished (w, m) block out immediately
                nc.sync.dma_start(
                    o3[:, blk * WB:(blk + 1) * WB, mh * MH:(mh + 1) * MH],
                    oa3[:, blk * WB:(blk + 1) * WB, :])
                yield

            def drain(g):
                if g is not None:
                    for _ in g:
                        pass

            # software pipeline with fine-grained emission interleave:
            # stage1(k+1) j0-groups alternate with stage2(k) matmul triples so
            # the in-order PE queue has s2 work between s1 psum-bank reuses.
            tiles = [(mh, blk) for mh in range(2) for blk in range(NBLK)]
            oaccs = {}
            for mh in range(2):
                oacc = opool.tile([CO, W * MH], f32, tag=f"oacc{mh}")
                oaccs[mh] = oacc[:].rearrange("p (w m) -> p w m", m=MH)
            hold = []
            g = stage1_gen(*tiles[0], hold)
            drain(g)
            pend = hold[0]
            for k, (mh, blk) in enumerate(tiles):
                s2g = stage2_gen(pend, oaccs[mh], mh, blk)
                if k + 1 < len(tiles):
                    hold = []
                    s1g = stage1_gen(*tiles[k + 1], hold)
                    _END = object()
                    while True:
                        a = next(s1g, _END)
                        b = next(s2g, _END)
                        if a is _END and b is _END:
                            break
                    pend = hold[0]
                else:
                    drain(s2g)
    nc.compile()
    return nc


def _get_prog():
    global _PROG
    if _PROG is None:
        _PROG = _build_program()
    return _PROG


def _build_wstack(weight):
    # wst[(d,i), (t, c, o)]: c<2: d=0 -> s=2c, d=1 -> s=2c+1 (same tap t)
    # c==2: d=0 -> (s=4, tap t), d=1 -> (s=4, tap t+1)  [t-pair packing]
    wst = np.zeros((128, KW * NSLOT * CO), np.float32)
    for t in range(KW):
        for c in range(NSLOT):
            col = (t * NSLOT + c) * CO
            wst[0:64, col:col + CO] = weight[:, :, 2 * c, t].T
            if c < 2:
                wst[64:128, col:col + CO] = weight[:, :, 2 * c + 1, t].T
            elif t + 1 < KW:
                wst[64:128, col:col + CO] = weight[:, :, 4, t + 1].T
    return np.ascontiguousarray(wst).astype(_np_dt())


def kernel(x, weight, bias):
    from concourse.bass_utils import run_bass_kernel_spmd

    global _CONSTS
    if _CONSTS is None:
        _CONSTS = _build_consts()
    GT = _CONSTS

    x = np.ascontiguousarray(np.asarray(x, dtype=np.float32))
    weight = np.ascontiguousarray(np.asarray(weight, dtype=np.float32))
    bias = np.ascontiguousarray(np.asarray(bias, dtype=np.float32))

    wst = _build_wstack(weight)
    b2 = np.ascontiguousarray(bias.reshape(CO, 1))

    in_maps = []
    for b in range(B):
        # layout [h, (w', d, i)]: per w-column the [h, (d,i)] lhsT is
        # contiguous; w' rolled by HALO so stage-1 halo reads don't wrap
        # until the last block (lets chunked DMA arrive in use order).
        xt = np.ascontiguousarray(x[b].transpose(1, 2, 0))        # [h, w, i]
        xt = np.roll(xt, HALO, axis=1)                            # [h, w', i]
        xdup = np.ascontiguousarray(
            np.broadcast_to(xt[:, :, None, :], (H, W, 2, CI))
        ).reshape(H, W * 2 * CI).astype(_np_dt())
        in_maps.append({"x": xdup, "g": GT, "wt": wst, "bias": b2})

    res = run_bass_kernel_spmd(_get_prog(), in_maps, core_ids=list(range(B)),
                               **_RUN_OPTS)
    global _LAST_RESULT
    _LAST_RESULT = res
    # out is stored (w, m)-major on device: [CO, W, H] -> [CO, H, W]
    out = np.stack([res.results[b]["out"].reshape(CO, W, H).transpose(0, 2, 1)
                    for b in range(B)], axis=0)
    return np.ascontiguousarray(out.astype(np.float32))



# revision 15
# speedup vs baseline: 1.2181x; 1.2181x over previous
"""Trainium2 Bass kernel for nn_CCL__69277822485245 (spectral conv via DCT/FFT).

Math: the reference's rFFT along W cancels into a circular 5-tap convolution,
and the DCT-II sandwich M @ diag(D[:,s]) @ D collapses into 5 dense 128x128
matrices G_s (precomputed on host). Per batch element:

    u_s[i, m, w] = sum_h G_s[m, h] x[i, h, w]                  (stage 1)
    out[o, m, n] = sum_{s,t,i} W[o,i,s,t] u_s[i, m, (n-t)%W] + bias[o]   (stage 2)

Sharding: data-parallel over batch B=8 across the 8 NeuronCores (1 each).

Layouts (per core):
  stage 1: per output column w, one matmul
      lhsT = xdup[h=128, di=128]    (x duplicated on the host so the output
                                     partition dim carries (d, i) pairs)
      rhs  = G^T[h=128, (s5, m64)]  (m in halves of 64 -> N=320; G s-order
                                     is [0,2,4,1,3] so each half's psum->u
                                     copy is a contiguous column slice)
      out  = psum[(d,i)=128, (sidx, m)]
      psum->SBUF casts split the halves: partitions 0-63 keep s={0,2,4}
      (slots 0..2), partitions 64-127 keep s={1,3} (slots 0..1), batched
      two w-columns per cast. SBUF u[(d,i), (slot, j, m)] -- j-major-of-m
      so stage-2 reads contiguous (j,m) runs.
  stage 2: for each t (same shift for both halves) and slot c:
      one K=128 matmul contracts (i, s=2c) on partitions 0-63 and
      (i, s=2c+1) on 64-127 simultaneously (c=2: K=64, s=4 only);
      15 sequential PSUM-accumulating passes, N = (j8, m64) = 512 contiguous.
      Bias added during the single per-block PSUM->SBUF evac (ScalarE).

DTYPE selects the matmul operand precision:
  "bf16": fastest (1 cyc/row + fast weight load), rel err ~ 3e-3
  "f32r": TF32-like (~2 cyc/row), rel err ~ 2e-4
  "f32" : exact fp32 (4 cyc/row), slowest
"""

import numpy as np

H = 128
W = 128
CI = 64
CO = 128
KH = 5
KW = 5
B = 8

MH = 64          # m-half processed per outer iteration
WB = 16          # w-block
HALO = 4         # extra back-columns for the t-shifts
WEXT = WB + HALO
NSLOT = 3        # s-slots per partition half (s = 2c + d)
JT = 8           # j-tile inside stage 2 (N = JT*MH = 512)

DTYPE = "bf16"

_PROG = None
_CONSTS = None
_RUN_OPTS = {}     # test harness may set e.g. {"trace": True, "trace_cores": [0]}
_LAST_RESULT = None


def _np_dt():
    if DTYPE == "bf16":
        import ml_dtypes
        return ml_dtypes.bfloat16
    return np.float32


def _build_consts():
    n = np.arange(H, dtype=np.float64)
    ang = np.pi * (2.0 * n[None, :] + 1.0) * n[:, None] / (2.0 * H)  # [k, h]
    D = 2.0 * np.cos(ang)
    wgt = np.where(n == 0, 0.5, 1.0)
    M = (np.cos(ang).T * wgt[None, :]) / (2.0 * H)                    # [m, k]
    G = np.stack([M @ (D[:, s:s + 1] * D) for s in range(KH)])        # [s, m, h]
    G = G[[0, 2, 4, 1, 3]]   # s-order so each half's psum->u copy is contiguous
    # rhs layout [h, (mh, sidx, ml)]: col = mh*320 + sidx*64 + ml
    GT = (G.transpose(2, 0, 1)                # [h, s, m]
            .reshape(H, KH, 2, MH)            # [h, s, mh, ml]
            .transpose(0, 2, 1, 3)            # [h, mh, s, ml]
            .reshape(H, KH * H))
    return np.ascontiguousarray(GT).astype(_np_dt())


def _build_program():
    import concourse.mybir as mybir
    import concourse.tile as tile
    from concourse import bacc

    f32 = mybir.dt.float32
    mmdt = {"bf16": mybir.dt.bfloat16,
            "f32r": mybir.dt.float32r,
            "f32": mybir.dt.float32}[DTYPE]

    nc = bacc.Bacc("TRN2", target_bir_lowering=False, debug=False,
                   enable_asserts=False, num_devices=B)
    NBLK = W // WB
    # x stored rolled by HALO along w (stored col s holds w = s - HALO mod W)
    # and chunked so stage 1 can start before the full x has landed.
    x_d = nc.dram_tensor("x", [H, W * 2 * CI], mmdt, kind="ExternalInput").ap()
    g_d = nc.dram_tensor("g", [H, KH * H], mmdt, kind="ExternalInput").ap()
    w_d = nc.dram_tensor("wt", [128, KW * NSLOT * CO], mmdt,
                         kind="ExternalInput").ap()
    b_d = nc.dram_tensor("bias", [CO, 1], f32, kind="ExternalInput").ap()
    # out stored (w, m)-major; host transposes back to (m, w)
    o_d = nc.dram_tensor("out", [CO, W * H], f32, kind="ExternalOutput").ap()
    o3 = o_d.rearrange("p (w m) -> p w m", m=H)

    with tile.TileContext(nc) as tc:
        with (
            tc.tile_pool(name="const", bufs=1) as cpool,
            tc.tile_pool(name="xch", bufs=1) as xpool,
            tc.tile_pool(name="u", bufs=2) as upool,
            tc.tile_pool(name="oacc", bufs=1) as opool,
            tc.tile_pool(name="ps1", bufs=2, space="PSUM") as ps1,
            tc.tile_pool(name="ps2", bufs=2, space="PSUM") as ps2,
        ):
            gt = cpool.tile([H, KH * H], mmdt)
            nc.sync.dma_start(gt[:], g_d)
            wt = cpool.tile([128, KW * NSLOT * CO], mmdt)
            nc.scalar.dma_start(wt[:], w_d)
            bt = cpool.tile([CO, 1], f32)
            nc.scalar.dma_start(bt[:], b_d)
            x_c = x_d.rearrange("p (b rest) -> p b rest", b=NBLK)
            xch = []
            for bk in range(NBLK):
                xc = xpool.tile([H, WB * 2 * CI], mmdt, tag=f"x{bk}")
                # alternate the two HWDGE queues so triggers + transfers overlap
                eng = nc.sync if bk % 2 == 0 else nc.scalar
                eng.dma_start(xc[:], x_c[:, bk, :])
                xch.append(xc[:].rearrange("p (w di) -> p w di", w=WB))

            import concourse.mybir as _mb

            def stage1_gen(mh, blk, out):
                """Yields after each j0-group (2 matmuls + 2 evac copies)."""
                u = upool.tile([128, NSLOT * WEXT * MH], mmdt)
                u4 = u[:].rearrange("p (c j m) -> p c j m", c=NSLOT, j=WEXT)
                out.append(u4)
                for j0 in range(0, WEXT, 2):
                    p1 = ps1.tile([128, 1024], f32)
                    for dj in range(2):
                        sc = (blk * WB + j0 + dj) % W   # stored col (pre-rolled)
                        nc.tensor.matmul(p1[:, dj * 512:dj * 512 + KH * MH],
                                         xch[sc // WB][:, sc % WB, :],
                                         gt[:, mh * KH * MH:(mh + 1) * KH * MH],
                                         start=True, stop=True)
                    pv = p1[:].rearrange("p (j s m) -> p j s m", j=2, s=8)
                    # psum s-order [0,2,4,1,3]: half0 cols 0:192, half1 192:320
                    # evac split across engines: DVE (half0) + Act (half1)
                    nc.vector.tensor_copy(
                        u4[0:64, :, j0:j0 + 2, :].transpose([0, 2, 1, 3]),
                        pv[0:64, :, 0:3, :])
                    nc.scalar.activation(
                        u4[64:128, 0:2, j0:j0 + 2, :].transpose([0, 2, 1, 3]),
                        pv[64:128, :, 3:5, :],
                        _mb.ActivationFunctionType.Identity)
                    # replicate s=4 into the idle half-1 slot shifted one w
                    # back (u4[64:128, 2, j] := u_s4[i, j-1]) via SBUF-to-SBUF
                    # DMA on the scalar HWDGE queue (cross-partition move; two
                    # halves per tile so the data lands a full phase before
                    # the c=2 t-pair matmuls read it).
                    if j0 == 8:
                        nc.scalar.dma_start(u4[64:128, 2, 1:11, :],
                                            u4[0:64, 2, 0:10, :])
                    elif j0 == WEXT - 2:
                        nc.scalar.dma_start(u4[64:128, 2, 11:WEXT, :],
                                            u4[0:64, 2, 10:WEXT - 1, :])
                    yield

            # c=2 runs last so the s4-shift DMA has landed; taps paired
            # (t, t+1) for t in {0, 2}, t=4 alone at K=64.
            S2BLOCKS = ([(t, c, 128) for c in (0, 1) for t in range(KW)]
                        + [(0, 2, 128), (2, 2, 128), (4, 2, 64)])

            def stage2_gen(u4, oa3, mh, blk):
                """Yields after every 3 matmuls."""
                p2 = ps2.tile([128, 2 * 512], f32)
                nmm = 0
                for bi, (t, c, kk) in enumerate(S2BLOCKS):
                    lhsT2 = wt[0:kk, (t * NSLOT + c) * CO:
                               (t * NSLOT + c + 1) * CO]
                    start = bi == 0
                    stop = bi == len(S2BLOCKS) - 1
                    for jt in range(WB // JT):
                        # contiguous (j8, m64) = 512 elems
                        rhs2 = u4[0:kk, c,
                                  HALO - t + jt * JT:
                                  HALO - t + (jt + 1) * JT, :]
                        nc.tensor.matmul(
                            p2[:, jt * 512:(jt + 1) * 512], lhsT2, rhs2,
                            start=start, stop=stop)
                        nmm += 1
                        if nmm % 3 == 0:
                            yield
                # psum (jt, j, m) == (w, m) order matches oacc layout: no
                # transpose in the evac; alternate engines by tile parity so
                # the 1.1us evac doesn't bunch either queue at tile boundaries
                if (mh * NBLK + blk) % 2 == 0:
                    nc.vector.tensor_scalar_add(
                        oa3[:, blk * WB:(blk + 1) * WB, :],
                        p2[:].rearrange("p (w m) -> p w m", m=MH),
                        bt[:])
                else:
                    nc.scalar.activation(
                        oa3[:, blk * WB:(blk + 1) * WB, :],
                        p2[:].rearrange("p (w m) -> p w m", m=MH),
                        _mb.ActivationFunctionType.Identity, bias=bt[:])
                # stream this tile's finished (w, m) block out immediately
                nc.sync.dma_start(
                    o3[:, blk * WB:(blk + 1) * WB, mh * MH:(mh + 1) * MH],
                    oa3[:, blk * WB:(blk + 1) * WB, :])
                yield

            def drain(g):
                if g is not None:
                    for _ in g:
                        pass

            # software pipeline with fine-grained emission interleave:
            # stage1(k+1) j0-groups alternate with stage2(k) matmul triples so
            # the in-order PE queue has s2 work between s1 psum-bank reuses.
            tiles = [(mh, blk) for mh in range(2) for blk in range(NBLK)]
            oaccs = {}
            for mh in range(2):
                oacc = opool.tile([CO, W * MH], f32, tag=f"oacc{mh}")
                oaccs[mh] = oacc[:].rearrange("p (w m) -> p w m", m=MH)
            hold = []
            g = stage1_gen(*tiles[0], hold)
            drain(g)
            pend = hold[0]
            for k, (mh, blk) in enumerate(tiles):
                s2g = stage2_gen(pend, oaccs[mh], mh, blk)
                if k + 1 < len(tiles):
                    hold = []
                    s1g = stage1_gen(*tiles[k + 1], hold)
                    _END = object()
                    while True:
                        a = next(s1g, _END)
                        b = next(s2g, _END)
                        if a is _END and b is _END:
                            break
                    pend = hold[0]
                else:
                    drain(s2g)
    nc.compile()
    return nc


def _get_prog():
    global _PROG
    if _PROG is None:
        _PROG = _build_program()
    return _PROG


def _build_wstack(weight):
    # wst[(d,i), (t, c, o)]: c<2: d=0 -> s=2c, d=1 -> s=2c+1 (same tap t)
    # c==2: d=0 -> (s=4, tap t), d=1 -> (s=4, tap t+1)  [t-pair packing]
    wst = np.zeros((128, KW * NSLOT * CO), np.float32)
    for t in range(KW):
        for c in range(NSLOT):
            col = (t * NSLOT + c) * CO
            wst[0:64, col:col + CO] = weight[:, :, 2 * c, t].T
            if c < 2:
                wst[64:128, col:col + CO] = weight[:, :, 2 * c + 1, t].T
            elif t + 1 < KW:
                wst[64:128, col:col + CO] = weight[:, :, 4, t + 1].T
    return np.ascontiguousarray(wst).astype(_np_dt())


def kernel(x, weight, bias):
    from concourse.bass_utils import run_bass_kernel_spmd

    global _CONSTS
    if _CONSTS is None:
        _CONSTS = _build_consts()
    GT = _CONSTS

    x = np.ascontiguousarray(np.asarray(x, dtype=np.float32))
    weight = np.ascontiguousarray(np.asarray(weight, dtype=np.float32))
    bias = np.ascontiguousarray(np.asarray(bias, dtype=np.float32))

    wst = _build_wstack(weight)
    b2 = np.ascontiguousarray(bias.reshape(CO, 1))

    in_maps = []
    for b in range(B):
        # layout [h, (w', d, i)]: per w-column the [h, (d,i)] lhsT is
        # contiguous; w' rolled by HALO so stage-1 halo reads don't wrap
        # until the last block (lets chunked DMA arrive in use order).
        xt = np.ascontiguousarray(x[b].transpose(1, 2, 0))        # [h, w, i]
        xt = np.roll(xt, HALO, axis=1)                            # [h, w', i]
        xdup = np.ascontiguousarray(
            np.broadcast_to(xt[:, :, None, :], (H, W, 2, CI))
        ).reshape(H, W * 2 * CI).astype(_np_dt())
        in_maps.append({"x": xdup, "g": GT, "wt": wst, "bias": b2})

    res = run_bass_kernel_spmd(_get_prog(), in_maps, core_ids=list(range(B)),
                               **_RUN_OPTS)
    global _LAST_RESULT
    _LAST_RESULT = res
    # out is stored (w, m)-major on device: [CO, W, H] -> [CO, H, W]
    out = np.stack([res.results[b]["out"].reshape(CO, W, H).transpose(0, 2, 1)
                    for b in range(B)], axis=0)
    return np.ascontiguousarray(out.astype(np.float32))



# revision 16
# speedup vs baseline: 1.2442x; 1.0214x over previous
"""Trainium2 Bass kernel for nn_CCL__69277822485245 (spectral conv via DCT/FFT).

Math: the reference's rFFT along W cancels into a circular 5-tap convolution,
and the DCT-II sandwich M @ diag(D[:,s]) @ D collapses into 5 dense 128x128
matrices G_s (precomputed on host). Per batch element:

    u_s[i, m, w] = sum_h G_s[m, h] x[i, h, w]                  (stage 1)
    out[o, m, n] = sum_{s,t,i} W[o,i,s,t] u_s[i, m, (n-t)%W] + bias[o]   (stage 2)

Sharding: data-parallel over batch B=8 across the 8 NeuronCores (1 each).

Layouts (per core):
  stage 1: per output column w, one matmul
      lhsT = xdup[h=128, di=128]    (x duplicated on the host so the output
                                     partition dim carries (d, i) pairs)
      rhs  = G^T[h=128, (s5, m64)]  (m in halves of 64 -> N=320; G s-order
                                     is [0,2,4,1,3] so each half's psum->u
                                     copy is a contiguous column slice)
      out  = psum[(d,i)=128, (sidx, m)]
      psum->SBUF casts split the halves: partitions 0-63 keep s={0,2,4}
      (slots 0..2), partitions 64-127 keep s={1,3} (slots 0..1), batched
      two w-columns per cast. SBUF u[(d,i), (slot, j, m)] -- j-major-of-m
      so stage-2 reads contiguous (j,m) runs.
  stage 2: for each t (same shift for both halves) and slot c:
      one K=128 matmul contracts (i, s=2c) on partitions 0-63 and
      (i, s=2c+1) on 64-127 simultaneously (c=2: K=64, s=4 only);
      15 sequential PSUM-accumulating passes, N = (j8, m64) = 512 contiguous.
      Bias added during the single per-block PSUM->SBUF evac (ScalarE).

DTYPE selects the matmul operand precision:
  "bf16": fastest (1 cyc/row + fast weight load), rel err ~ 3e-3
  "f32r": TF32-like (~2 cyc/row), rel err ~ 2e-4
  "f32" : exact fp32 (4 cyc/row), slowest
"""

import numpy as np

H = 128
W = 128
CI = 64
CO = 128
KH = 5
KW = 5
B = 8

MH = 64          # m-half processed per outer iteration
WB = 16          # w-block
HALO = 4         # extra back-columns for the t-shifts
WEXT = WB + HALO
NSLOT = 3        # s-slots per partition half (s = 2c + d)
JT = 8           # j-tile inside stage 2 (N = JT*MH = 512)

DTYPE = "bf16"

_PROG = None
_CONSTS = None
_RUN_OPTS = {}     # test harness may set e.g. {"trace": True, "trace_cores": [0]}
_LAST_RESULT = None


def _np_dt():
    if DTYPE == "bf16":
        import ml_dtypes
        return ml_dtypes.bfloat16
    return np.float32


def _build_consts():
    n = np.arange(H, dtype=np.float64)
    ang = np.pi * (2.0 * n[None, :] + 1.0) * n[:, None] / (2.0 * H)  # [k, h]
    D = 2.0 * np.cos(ang)
    wgt = np.where(n == 0, 0.5, 1.0)
    M = (np.cos(ang).T * wgt[None, :]) / (2.0 * H)                    # [m, k]
    G = np.stack([M @ (D[:, s:s + 1] * D) for s in range(KH)])        # [s, m, h]
    G = G[[0, 2, 4, 1, 3]]   # s-order so each half's psum->u copy is contiguous
    # rhs layout [h, (mh, sidx, ml)]: col = mh*320 + sidx*64 + ml
    GT = (G.transpose(2, 0, 1)                # [h, s, m]
            .reshape(H, KH, 2, MH)            # [h, s, mh, ml]
            .transpose(0, 2, 1, 3)            # [h, mh, s, ml]
            .reshape(H, KH * H))
    return np.ascontiguousarray(GT).astype(_np_dt())


def _build_program():
    import concourse.mybir as mybir
    import concourse.tile as tile
    from concourse import bacc

    f32 = mybir.dt.float32
    mmdt = {"bf16": mybir.dt.bfloat16,
            "f32r": mybir.dt.float32r,
            "f32": mybir.dt.float32}[DTYPE]

    nc = bacc.Bacc("TRN2", target_bir_lowering=False, debug=False,
                   enable_asserts=False, num_devices=B)
    NBLK = W // WB
    # x stored rolled by HALO along w (stored col s holds w = s - HALO mod W)
    # and chunked so stage 1 can start before the full x has landed.
    x_d = nc.dram_tensor("x", [H, W * 2 * CI], mmdt, kind="ExternalInput").ap()
    g_d = nc.dram_tensor("g", [H, KH * H], mmdt, kind="ExternalInput").ap()
    w_d = nc.dram_tensor("wt", [128, KW * NSLOT * CO], mmdt,
                         kind="ExternalInput").ap()
    b_d = nc.dram_tensor("bias", [CO, 1], f32, kind="ExternalInput").ap()
    # out stored (w, m)-major; host transposes back to (m, w)
    o_d = nc.dram_tensor("out", [CO, W * H], f32, kind="ExternalOutput").ap()
    o3 = o_d.rearrange("p (w m) -> p w m", m=H)

    with tile.TileContext(nc) as tc:
        with (
            tc.tile_pool(name="const", bufs=1) as cpool,
            tc.tile_pool(name="xch", bufs=1) as xpool,
            tc.tile_pool(name="u", bufs=3) as upool,
            tc.tile_pool(name="oacc", bufs=1) as opool,
            tc.tile_pool(name="ps1", bufs=2, space="PSUM") as ps1,
            tc.tile_pool(name="ps2", bufs=2, space="PSUM") as ps2,
        ):
            gt = cpool.tile([H, KH * H], mmdt)
            nc.sync.dma_start(gt[:], g_d)
            wt = cpool.tile([128, KW * NSLOT * CO], mmdt)
            nc.scalar.dma_start(wt[:], w_d)
            bt = cpool.tile([CO, 1], f32)
            nc.scalar.dma_start(bt[:], b_d)
            x_c = x_d.rearrange("p (b rest) -> p b rest", b=NBLK)
            xch = []
            for bk in range(NBLK):
                xc = xpool.tile([H, WB * 2 * CI], mmdt, tag=f"x{bk}")
                # alternate the two HWDGE queues so triggers + transfers overlap
                eng = nc.sync if bk % 2 == 0 else nc.scalar
                eng.dma_start(xc[:], x_c[:, bk, :])
                xch.append(xc[:].rearrange("p (w di) -> p w di", w=WB))

            import concourse.mybir as _mb

            def stage1_gen(mh, blk, out):
                """Yields after each j0-group (2 matmuls + 2 evac copies)."""
                u = upool.tile([128, NSLOT * WEXT * MH], mmdt)
                u4 = u[:].rearrange("p (c j m) -> p c j m", c=NSLOT, j=WEXT)
                out.append(u4)
                for j0 in range(0, WEXT, 2):
                    p1 = ps1.tile([128, 1024], f32)
                    for dj in range(2):
                        sc = (blk * WB + j0 + dj) % W   # stored col (pre-rolled)
                        nc.tensor.matmul(p1[:, dj * 512:dj * 512 + KH * MH],
                                         xch[sc // WB][:, sc % WB, :],
                                         gt[:, mh * KH * MH:(mh + 1) * KH * MH],
                                         start=True, stop=True)
                    pv = p1[:].rearrange("p (j s m) -> p j s m", j=2, s=8)
                    # psum s-order [0,2,4,1,3]: half0 cols 0:192, half1 192:320
                    # evac split across engines: DVE (half0) + Act (half1)
                    nc.vector.tensor_copy(
                        u4[0:64, :, j0:j0 + 2, :].transpose([0, 2, 1, 3]),
                        pv[0:64, :, 0:3, :])
                    nc.scalar.activation(
                        u4[64:128, 0:2, j0:j0 + 2, :].transpose([0, 2, 1, 3]),
                        pv[64:128, :, 3:5, :],
                        _mb.ActivationFunctionType.Identity)
                    # replicate s=4 into the idle half-1 slot shifted one w
                    # back (u4[64:128, 2, j] := u_s4[i, j-1]) via SBUF-to-SBUF
                    # DMA on the scalar HWDGE queue (cross-partition move; two
                    # halves per tile so the data lands a full phase before
                    # the c=2 t-pair matmuls read it).
                    if j0 == 8:
                        nc.scalar.dma_start(u4[64:128, 2, 1:11, :],
                                            u4[0:64, 2, 0:10, :])
                    elif j0 == WEXT - 2:
                        nc.scalar.dma_start(u4[64:128, 2, 11:WEXT, :],
                                            u4[0:64, 2, 10:WEXT - 1, :])
                    yield

            # c=2 runs last so the s4-shift DMA has landed; taps paired
            # (t, t+1) for t in {0, 2}, t=4 alone at K=64.
            S2BLOCKS = ([(t, c, 128) for c in (0, 1) for t in range(KW)]
                        + [(0, 2, 128), (2, 2, 128), (4, 2, 64)])

            def stage2_gen(u4, oa3, mh, blk):
                """Yields after every 3 matmuls."""
                p2 = ps2.tile([128, 2 * 512], f32)
                nmm = 0
                for bi, (t, c, kk) in enumerate(S2BLOCKS):
                    lhsT2 = wt[0:kk, (t * NSLOT + c) * CO:
                               (t * NSLOT + c + 1) * CO]
                    start = bi == 0
                    stop = bi == len(S2BLOCKS) - 1
                    for jt in range(WB // JT):
                        # contiguous (j8, m64) = 512 elems
                        rhs2 = u4[0:kk, c,
                                  HALO - t + jt * JT:
                                  HALO - t + (jt + 1) * JT, :]
                        nc.tensor.matmul(
                            p2[:, jt * 512:(jt + 1) * 512], lhsT2, rhs2,
                            start=start, stop=stop)
                        nmm += 1
                        if nmm % 3 == 0:
                            yield
                # psum (jt, j, m) == (w, m) order matches oacc layout: no
                # transpose in the evac; alternate engines by tile parity so
                # the 1.1us evac doesn't bunch either queue at tile boundaries
                if (mh * NBLK + blk) % 2 == 0:
                    nc.vector.tensor_scalar_add(
                        oa3[:, blk * WB:(blk + 1) * WB, :],
                        p2[:].rearrange("p (w m) -> p w m", m=MH),
                        bt[:])
                else:
                    nc.scalar.activation(
                        oa3[:, blk * WB:(blk + 1) * WB, :],
                        p2[:].rearrange("p (w m) -> p w m", m=MH),
                        _mb.ActivationFunctionType.Identity, bias=bt[:])
                # stream this tile's finished (w, m) block out immediately
                nc.sync.dma_start(
                    o3[:, blk * WB:(blk + 1) * WB, mh * MH:(mh + 1) * MH],
                    oa3[:, blk * WB:(blk + 1) * WB, :])
                yield

            def drain(g):
                if g is not None:
                    for _ in g:
                        pass

            # software pipeline with fine-grained emission interleave:
            # stage1(k+1) j0-groups alternate with stage2(k) matmul triples so
            # the in-order PE queue has s2 work between s1 psum-bank reuses.
            tiles = [(mh, blk) for mh in range(2) for blk in range(NBLK)]
            oaccs = {}
            for mh in range(2):
                oacc = opool.tile([CO, W * MH], f32, tag=f"oacc{mh}")
                oaccs[mh] = oacc[:].rearrange("p (w m) -> p w m", m=MH)
            hold = []
            g = stage1_gen(*tiles[0], hold)
            drain(g)
            pend = hold[0]
            for k, (mh, blk) in enumerate(tiles):
                s2g = stage2_gen(pend, oaccs[mh], mh, blk)
                if k + 1 < len(tiles):
                    hold = []
                    s1g = stage1_gen(*tiles[k + 1], hold)
                    _END = object()
                    while True:
                        a = next(s1g, _END)
                        b = next(s2g, _END)
                        if a is _END and b is _END:
                            break
                    pend = hold[0]
                else:
                    drain(s2g)
    nc.compile()
    return nc


def _get_prog():
    global _PROG
    if _PROG is None:
        _PROG = _build_program()
    return _PROG


def _build_wstack(weight):
    # wst[(d,i), (t, c, o)]: c<2: d=0 -> s=2c, d=1 -> s=2c+1 (same tap t)
    # c==2: d=0 -> (s=4, tap t), d=1 -> (s=4, tap t+1)  [t-pair packing]
    wst = np.zeros((128, KW * NSLOT * CO), np.float32)
    for t in range(KW):
        for c in range(NSLOT):
            col = (t * NSLOT + c) * CO
            wst[0:64, col:col + CO] = weight[:, :, 2 * c, t].T
            if c < 2:
                wst[64:128, col:col + CO] = weight[:, :, 2 * c + 1, t].T
            elif t + 1 < KW:
                wst[64:128, col:col + CO] = weight[:, :, 4, t + 1].T
    return np.ascontiguousarray(wst).astype(_np_dt())


def kernel(x, weight, bias):
    from concourse.bass_utils import run_bass_kernel_spmd

    global _CONSTS
    if _CONSTS is None:
        _CONSTS = _build_consts()
    GT = _CONSTS

    x = np.ascontiguousarray(np.asarray(x, dtype=np.float32))
    weight = np.ascontiguousarray(np.asarray(weight, dtype=np.float32))
    bias = np.ascontiguousarray(np.asarray(bias, dtype=np.float32))

    wst = _build_wstack(weight)
    b2 = np.ascontiguousarray(bias.reshape(CO, 1))

    in_maps = []
    for b in range(B):
        # layout [h, (w', d, i)]: per w-column the [h, (d,i)] lhsT is
        # contiguous; w' rolled by HALO so stage-1 halo reads don't wrap
        # until the last block (lets chunked DMA arrive in use order).
        xt = np.ascontiguousarray(x[b].transpose(1, 2, 0))        # [h, w, i]
        xt = np.roll(xt, HALO, axis=1)                            # [h, w', i]
        xdup = np.ascontiguousarray(
            np.broadcast_to(xt[:, :, None, :], (H, W, 2, CI))
        ).reshape(H, W * 2 * CI).astype(_np_dt())
        in_maps.append({"x": xdup, "g": GT, "wt": wst, "bias": b2})

    res = run_bass_kernel_spmd(_get_prog(), in_maps, core_ids=list(range(B)),
                               **_RUN_OPTS)
    global _LAST_RESULT
    _LAST_RESULT = res
    # out is stored (w, m)-major on device: [CO, W, H] -> [CO, H, W]
    out = np.stack([res.results[b]["out"].reshape(CO, W, H).transpose(0, 2, 1)
                    for b in range(B)], axis=0)
    return np.ascontiguousarray(out.astype(np.float32))



# revision 17
# speedup vs baseline: 1.4481x; 1.1638x over previous
"""Trainium2 Bass kernel for nn_CCL__69277822485245 (spectral conv via DCT/FFT).

Math: the reference's rFFT along W cancels into a circular 5-tap convolution,
and the DCT-II sandwich M @ diag(D[:,s]) @ D collapses into 5 dense 128x128
matrices G_s (precomputed on host). Per batch element:

    u_s[i, m, w] = sum_h G_s[m, h] x[i, h, w]                  (stage 1)
    out[o, m, n] = sum_{s,t,i} W[o,i,s,t] u_s[i, m, (n-t)%W] + bias[o]   (stage 2)

Sharding: data-parallel over batch B=8 across the 8 NeuronCores (1 each).

Layouts (per core):
  stage 1: per output column w, one matmul
      lhsT = xdup[h=128, di=128]    (x duplicated on the host so the output
                                     partition dim carries (d, i) pairs)
      rhs  = G^T[h=128, (s5, m64)]  (m in halves of 64 -> N=320; G s-order
                                     is [0,2,4,1,3] so each half's psum->u
                                     copy is a contiguous column slice)
      out  = psum[(d,i)=128, (sidx, m)]
      psum->SBUF casts split the halves: partitions 0-63 keep s={0,2,4}
      (slots 0..2), partitions 64-127 keep s={1,3} (slots 0..1), batched
      two w-columns per cast. SBUF u[(d,i), (slot, j, m)] -- j-major-of-m
      so stage-2 reads contiguous (j,m) runs.
  stage 2: for each t (same shift for both halves) and slot c:
      one K=128 matmul contracts (i, s=2c) on partitions 0-63 and
      (i, s=2c+1) on 64-127 simultaneously (c=2: K=64, s=4 only);
      15 sequential PSUM-accumulating passes, N = (j8, m64) = 512 contiguous.
      Bias added during the single per-block PSUM->SBUF evac (ScalarE).

DTYPE selects the matmul operand precision:
  "bf16": fastest (1 cyc/row + fast weight load), rel err ~ 3e-3
  "f32r": TF32-like (~2 cyc/row), rel err ~ 2e-4
  "f32" : exact fp32 (4 cyc/row), slowest
"""

import numpy as np

H = 128
W = 128
CI = 64
CO = 128
KH = 5
KW = 5
B = 8

MH = 64          # m-half processed per outer iteration
WB = 16          # stage-2 w-block (psum-limited)
WB2 = 32         # stage-1 w-block (halo amortization, fewer phase boundaries)
HALO = 4         # extra back-columns for the t-shifts
WEXT = WB2 + HALO
NSLOT = 3        # s-slots per partition half (s = 2c + d)
JT = 8           # j-tile inside stage 2 (N = JT*MH = 512)

DTYPE = "bf16"

_PROG = None
_CONSTS = None
_RUN_OPTS = {}     # test harness may set e.g. {"trace": True, "trace_cores": [0]}
_LAST_RESULT = None


def _np_dt():
    if DTYPE == "bf16":
        import ml_dtypes
        return ml_dtypes.bfloat16
    return np.float32


def _build_consts():
    n = np.arange(H, dtype=np.float64)
    ang = np.pi * (2.0 * n[None, :] + 1.0) * n[:, None] / (2.0 * H)  # [k, h]
    D = 2.0 * np.cos(ang)
    wgt = np.where(n == 0, 0.5, 1.0)
    M = (np.cos(ang).T * wgt[None, :]) / (2.0 * H)                    # [m, k]
    G = np.stack([M @ (D[:, s:s + 1] * D) for s in range(KH)])        # [s, m, h]
    G = G[[0, 2, 4, 1, 3]]   # s-order so each half's psum->u copy is contiguous
    # rhs layout [h, (mh, sidx, ml)]: col = mh*320 + sidx*64 + ml
    GT = (G.transpose(2, 0, 1)                # [h, s, m]
            .reshape(H, KH, 2, MH)            # [h, s, mh, ml]
            .transpose(0, 2, 1, 3)            # [h, mh, s, ml]
            .reshape(H, KH * H))
    return np.ascontiguousarray(GT).astype(_np_dt())


def _build_program():
    import concourse.mybir as mybir
    import concourse.tile as tile
    from concourse import bacc

    f32 = mybir.dt.float32
    mmdt = {"bf16": mybir.dt.bfloat16,
            "f32r": mybir.dt.float32r,
            "f32": mybir.dt.float32}[DTYPE]

    nc = bacc.Bacc("TRN2", target_bir_lowering=False, debug=False,
                   enable_asserts=False, num_devices=B)
    NBLK = W // WB
    # x stored rolled by HALO along w (stored col s holds w = s - HALO mod W)
    # and chunked so stage 1 can start before the full x has landed.
    x_d = nc.dram_tensor("x", [H, W * 2 * CI], mmdt, kind="ExternalInput").ap()
    g_d = nc.dram_tensor("g", [H, KH * H], mmdt, kind="ExternalInput").ap()
    w_d = nc.dram_tensor("wt", [128, KW * NSLOT * CO], mmdt,
                         kind="ExternalInput").ap()
    b_d = nc.dram_tensor("bias", [CO, 1], f32, kind="ExternalInput").ap()
    # out stored (w, m)-major; host transposes back to (m, w)
    o_d = nc.dram_tensor("out", [CO, W * H], f32, kind="ExternalOutput").ap()
    o3 = o_d.rearrange("p (w m) -> p w m", m=H)

    with tile.TileContext(nc) as tc:
        with (
            tc.tile_pool(name="const", bufs=1) as cpool,
            tc.tile_pool(name="xch", bufs=1) as xpool,
            tc.tile_pool(name="u", bufs=2) as upool,
            tc.tile_pool(name="oacc", bufs=1) as opool,
            tc.tile_pool(name="ps1", bufs=2, space="PSUM") as ps1,
            tc.tile_pool(name="ps2", bufs=2, space="PSUM") as ps2,
        ):
            gt = cpool.tile([H, KH * H], mmdt)
            nc.sync.dma_start(gt[:], g_d)
            wt = cpool.tile([128, KW * NSLOT * CO], mmdt)
            nc.scalar.dma_start(wt[:], w_d)
            bt = cpool.tile([CO, 1], f32)
            nc.scalar.dma_start(bt[:], b_d)
            x_c = x_d.rearrange("p (b rest) -> p b rest", b=NBLK)
            xch = []
            for bk in range(NBLK):
                xc = xpool.tile([H, WB * 2 * CI], mmdt, tag=f"x{bk}")
                # alternate the two HWDGE queues so triggers + transfers overlap
                eng = nc.sync if bk % 2 == 0 else nc.scalar
                eng.dma_start(xc[:], x_c[:, bk, :])
                xch.append(xc[:].rearrange("p (w di) -> p w di", w=WB))

            import concourse.mybir as _mb

            def stage1_gen(mh, blk, out):
                """Yields after each j0-group (2 matmuls + 2 evac copies)."""
                u = upool.tile([128, NSLOT * WEXT * MH], mmdt)
                u4 = u[:].rearrange("p (c j m) -> p c j m", c=NSLOT, j=WEXT)
                out.append(u4)
                for j0 in range(0, WEXT, 2):
                    p1 = ps1.tile([128, 1024], f32)
                    for dj in range(2):
                        sc = (blk * WB2 + j0 + dj) % W  # stored col (pre-rolled)
                        nc.tensor.matmul(p1[:, dj * 512:dj * 512 + KH * MH],
                                         xch[sc // WB][:, sc % WB, :],
                                         gt[:, mh * KH * MH:(mh + 1) * KH * MH],
                                         start=True, stop=True)
                    pv = p1[:].rearrange("p (j s m) -> p j s m", j=2, s=8)
                    # psum s-order [0,2,4,1,3]: half0 cols 0:192, half1 192:320
                    # evac split across engines: DVE (half0) + Act (half1)
                    nc.vector.tensor_copy(
                        u4[0:64, :, j0:j0 + 2, :].transpose([0, 2, 1, 3]),
                        pv[0:64, :, 0:3, :])
                    nc.scalar.activation(
                        u4[64:128, 0:2, j0:j0 + 2, :].transpose([0, 2, 1, 3]),
                        pv[64:128, :, 3:5, :],
                        _mb.ActivationFunctionType.Identity)
                    # replicate s=4 into the idle half-1 slot shifted one w
                    # back (u4[64:128, 2, j] := u_s4[i, j-1]) via SBUF-to-SBUF
                    # DMA on the scalar HWDGE queue (cross-partition move; two
                    # halves per tile so the data lands well before the c=2
                    # t-pair matmuls read it).
                    if j0 == 16:
                        nc.scalar.dma_start(u4[64:128, 2, 1:17, :],
                                            u4[0:64, 2, 0:16, :])
                    elif j0 == WEXT - 2:
                        nc.scalar.dma_start(u4[64:128, 2, 17:WEXT, :],
                                            u4[0:64, 2, 16:WEXT - 1, :])
                    yield

            # c=2 runs last so the s4-shift DMA has landed; taps paired
            # (t, t+1) for t in {0, 2}, t=4 alone at K=64.
            S2BLOCKS = ([(t, c, 128) for c in (0, 1) for t in range(KW)]
                        + [(0, 2, 128), (2, 2, 128), (4, 2, 64)])

            def stage2_gen(u4, oa3, mh, blk):
                """One 16-wide output block (sub-block of a 32-wide stage-1
                tile); yields after every 3 matmuls."""
                joff = (blk % 2) * WB   # sub-block offset inside the u tile
                p2 = ps2.tile([128, 2 * 512], f32)
                nmm = 0
                for bi, (t, c, kk) in enumerate(S2BLOCKS):
                    lhsT2 = wt[0:kk, (t * NSLOT + c) * CO:
                               (t * NSLOT + c + 1) * CO]
                    start = bi == 0
                    stop = bi == len(S2BLOCKS) - 1
                    for jt in range(WB // JT):
                        # contiguous (j8, m64) = 512 elems
                        rhs2 = u4[0:kk, c,
                                  joff + HALO - t + jt * JT:
                                  joff + HALO - t + (jt + 1) * JT, :]
                        nc.tensor.matmul(
                            p2[:, jt * 512:(jt + 1) * 512], lhsT2, rhs2,
                            start=start, stop=stop)
                        nmm += 1
                        if nmm % 3 == 0:
                            yield
                # psum (jt, j, m) == (w, m) order matches oacc layout: no
                # transpose in the evac; alternate engines by tile parity so
                # the 1.1us evac doesn't bunch either queue at tile boundaries
                if (mh * NBLK + blk) % 2 == 0:
                    nc.vector.tensor_scalar_add(
                        oa3[:, blk * WB:(blk + 1) * WB, :],
                        p2[:].rearrange("p (w m) -> p w m", m=MH),
                        bt[:])
                else:
                    nc.scalar.activation(
                        oa3[:, blk * WB:(blk + 1) * WB, :],
                        p2[:].rearrange("p (w m) -> p w m", m=MH),
                        _mb.ActivationFunctionType.Identity, bias=bt[:])
                # stream this tile's finished (w, m) block out immediately
                nc.sync.dma_start(
                    o3[:, blk * WB:(blk + 1) * WB, mh * MH:(mh + 1) * MH],
                    oa3[:, blk * WB:(blk + 1) * WB, :])
                yield

            def drain(g):
                if g is not None:
                    for _ in g:
                        pass

            # software pipeline with fine-grained emission interleave:
            # stage1(k+1) j0-groups alternate with stage2(k) matmul triples so
            # the in-order PE queue has s2 work between s1 psum-bank reuses.
            NB2 = W // WB2
            tiles = [(mh, b2) for mh in range(2) for b2 in range(NB2)]
            oaccs = {}
            for mh in range(2):
                oacc = opool.tile([CO, W * MH], f32, tag=f"oacc{mh}")
                oaccs[mh] = oacc[:].rearrange("p (w m) -> p w m", m=MH)

            def stage2_pair(u4, mh, b2):
                for sub in range(2):
                    blk = b2 * 2 + sub
                    yield from stage2_gen(u4, oaccs[mh], mh, blk)

            hold = []
            g = stage1_gen(*tiles[0], hold)
            drain(g)
            pend = hold[0]
            for k, (mh, b2) in enumerate(tiles):
                s2g = stage2_pair(pend, mh, b2)
                if k + 1 < len(tiles):
                    hold = []
                    s1g = stage1_gen(*tiles[k + 1], hold)
                    _END = object()
                    while True:
                        a = next(s1g, _END)
                        b = next(s2g, _END)
                        if a is _END and b is _END:
                            break
                    pend = hold[0]
                else:
                    drain(s2g)
    nc.compile()
    return nc


def _get_prog():
    global _PROG
    if _PROG is None:
        _PROG = _build_program()
    return _PROG


def _build_wstack(weight):
    # wst[(d,i), (t, c, o)]: c<2: d=0 -> s=2c, d=1 -> s=2c+1 (same tap t)
    # c==2: d=0 -> (s=4, tap t), d=1 -> (s=4, tap t+1)  [t-pair packing]
    wst = np.zeros((128, KW * NSLOT * CO), np.float32)
    for t in range(KW):
        for c in range(NSLOT):
            col = (t * NSLOT + c) * CO
            wst[0:64, col:col + CO] = weight[:, :, 2 * c, t].T
            if c < 2:
                wst[64:128, col:col + CO] = weight[:, :, 2 * c + 1, t].T
            elif t + 1 < KW:
                wst[64:128, col:col + CO] = weight[:, :, 4, t + 1].T
    return np.ascontiguousarray(wst).astype(_np_dt())


def kernel(x, weight, bias):
    from concourse.bass_utils import run_bass_kernel_spmd

    global _CONSTS
    if _CONSTS is None:
        _CONSTS = _build_consts()
    GT = _CONSTS

    x = np.ascontiguousarray(np.asarray(x, dtype=np.float32))
    weight = np.ascontiguousarray(np.asarray(weight, dtype=np.float32))
    bias = np.ascontiguousarray(np.asarray(bias, dtype=np.float32))

    wst = _build_wstack(weight)
    b2 = np.ascontiguousarray(bias.reshape(CO, 1))

    in_maps = []
    for b in range(B):
        # layout [h, (w', d, i)]: per w-column the [h, (d,i)] lhsT is
        # contiguous; w' rolled by HALO so stage-1 halo reads don't wrap
        # until the last block (lets chunked DMA arrive in use order).
        xt = np.ascontiguousarray(x[b].transpose(1, 2, 0))        # [h, w, i]
        xt = np.roll(xt, HALO, axis=1)                            # [h, w', i]
        xdup = np.ascontiguousarray(
            np.broadcast_to(xt[:, :, None, :], (H, W, 2, CI))
        ).reshape(H, W * 2 * CI).astype(_np_dt())
        in_maps.append({"x": xdup, "g": GT, "wt": wst, "bias": b2})

    res = run_bass_kernel_spmd(_get_prog(), in_maps, core_ids=list(range(B)),
                               **_RUN_OPTS)
    global _LAST_RESULT
    _LAST_RESULT = res
    # out is stored (w, m)-major on device: [CO, W, H] -> [CO, H, W]
    out = np.stack([res.results[b]["out"].reshape(CO, W, H).transpose(0, 2, 1)
                    for b in range(B)], axis=0)
    return np.ascontiguousarray(out.astype(np.float32))

